# revision 23
# baseline (speedup 1.0000x reference)
"""ARD RBF kernel matrix on 8 TRN2 NeuronCores.

out[n, m] = exp(log_outputscale) * exp(-0.5 * sum_d ((x[n,d] - y[m,d]) / l_d)^2)
with l = exp(log_lengthscale).

Per core (rows of x sharded 8-ways):
    invl[d]   = exp(-log_lengthscale[d])
    xs = x * invl, ys = y * invl        (rounded to fp32r, 11-bit mantissa)
    c[n, m]   = sum_d xs[n,d] * ys[m,d]            } one K=66 fp32r matmul:
    y2[m]     = -0.5 * sum_d ys[m,d]^2  (hi+lo rows)} lhsT=[xs; 1; 1]
    x2[n]     = -0.5 * sum_d xs[n,d]^2 + log_os     -> exact f32 ACT bias
    out[n, m] = Exp(c + y2 + x2)                    -> single ScalarE pass

fp32r streams at ~1 cycle/row (vs 4 for fp32) with 11-bit mantissa; the
y2 row (magnitude ~32) is stored as hi + residual-lo fp32r rows so its
rounding error stays ~1e-6. x2/log_os ride the activation bias in full
fp32. Measured HW rel err ~5e-4 (from the 11-bit rounding of xs/ys).

Inputs are staged host-side in transposed layout ([D, points]) so the
contraction dim lands on SBUF partitions with no on-device transposes.

Schedule shape (engines are in-order FIFOs, so program order matters):
y prep is emitted per column-half right before the main iterations that
consume it — otherwise the PE queue serializes ALL y2-prep matmuls ahead
of the first main matmul. Output DMAs are 2 MiB on the SP ring only (the
issuing sequencer babysits each transfer; keep the ACT ring free for the
exp epilogue). y2-prep matmuls borrow row 0 of main-pool PSUM tiles since
the main pool needs all 8 banks.
"""

import numpy as np

import concourse.bass as bass
import concourse.mybir as mybir
import concourse.tile as tile
from concourse import bacc
from concourse.bass_utils import run_bass_kernel_spmd

N_CORES = 8
N, M, D = 8192, 8192, 64
NSH = N // N_CORES  # 1024 x-rows per core

F32 = mybir.dt.float32
F32R = mybir.dt.float32r
AF = mybir.ActivationFunctionType


def build_nc(nsh=NSH, m=M, d=D, use_f32r=True, n_chunk=2048, prep_chunk=1024):
    """Per-core Bass graph. SPMD: same graph on all 8 cores."""
    nc = bacc.Bacc("TRN2", target_bir_lowering=False)

    xT = nc.dram_tensor("xT", [d, nsh], F32, kind="ExternalInput")
    yT = nc.dram_tensor("yT", [d, m], F32, kind="ExternalInput")
    lls = nc.dram_tensor("log_lengthscale", [d], F32, kind="ExternalInput")
    los = nc.dram_tensor("log_outputscale", [1], F32, kind="ExternalInput")
    out = nc.dram_tensor("out", [nsh, m], F32, kind="ExternalOutput")

    n_tiles = nsh // 128          # x tiles (output partition dim)
    mm_n = 512                    # moving free dim per matmul (one PSUM bank)
    n_sub = n_chunk // mm_n       # matmuls per ACT chunk
    naux = 2                      # y2 hi + lo rows
    half = m // 2                 # column half per outer iteration
    hc = half // n_chunk

    # fp32r: fp32 bits with the low 12 mantissa bits zeroed; streams at
    # ~1 cycle/row (vs 4 for fp32). Writers into matmul operands must
    # declare float32r output so HW rounds on write (BIR verifier rule).
    def mmi(ap):  # matmul input view
        return ap.bitcast(F32R) if use_f32r else ap

    def mmo(ap):  # rounded-writer output view
        return ap.bitcast(F32R) if use_f32r else ap

    with tile.TileContext(nc) as tc:
        with (
            tc.tile_pool(name="const", bufs=1) as cpool,
            tc.tile_pool(name="big", bufs=1) as bpool,
            tc.tile_pool(name="outp", bufs=2) as opool,
        ):
            # ---- hyperparameters ----
            lls_sb = cpool.tile([d, 1], F32, tag="lls")
            nc.sync.dma_start(out=lls_sb[:, :], in_=lls[:].rearrange("(d o) -> d o", o=1))
            los_sb = cpool.tile([1, 1], F32, tag="los")
            nc.sync.dma_start(out=los_sb[:, :], in_=los[:].rearrange("(a o) -> a o", o=1))

            invl = cpool.tile([d, 1], F32, tag="invl")
            nc.scalar.activation(invl[:, :], lls_sb[:, :], AF.Exp, scale=-1.0)
            # -0.5 weight vector for the square-reduce matmuls; consumed by
            # f32r matmuls so it needs an f32r-writing producer (copy).
            neghalf_f = cpool.tile([d, 1], F32, tag="neghalf_f")
            nc.vector.memset(neghalf_f[:, :], -0.5)
            neghalf = cpool.tile([d, 1], F32, tag="neghalf")
            nc.vector.tensor_copy(mmo(neghalf[:, :]), neghalf_f[:, :])
            ones11 = cpool.tile([1, 1], F32, tag="ones11")
            nc.vector.memset(ones11[:, :], 1.0)

            # ---- x side: x_aug = [xs; 1; 1] ----
            # Raw DMA lands in a separate tile: every writer of y_aug/x_aug
            # must carry float32r output dtype (BIR fp32r-producer rule).
            x_raw = bpool.tile([d, nsh], F32, tag="x_raw")
            nc.sync.dma_start(out=x_raw[:, :], in_=xT[:, :])
            x_aug = bpool.tile([d + naux, nsh], F32, tag="x_aug")
            nc.vector.tensor_scalar_mul(mmo(x_aug[0:d, :]), x_raw[:, :], invl[:, :])
            xsq = bpool.tile([d, nsh], F32, tag="xsq")
            nc.vector.tensor_mul(mmo(xsq[:, :]), x_aug[0:d, :], x_aug[0:d, :])
            # memset can't encode an f32r output dtype; copy from an f32 tile
            ones_rows = cpool.tile([naux, nsh], F32, tag="ones_rows")
            nc.vector.memset(ones_rows[:, :], 1.0)
            nc.vector.tensor_copy(mmo(x_aug[d : d + naux, :]), ones_rows[:, :])

            y_aug = bpool.tile([d + naux, m], F32, tag="y_aug")
            y2l_tmp = cpool.tile([1, m], F32, tag="y2l")
            x2row = cpool.tile([1, nsh], F32, tag="x2row")
            bias_sb = cpool.tile([128, n_tiles], F32, tag="bias")

            # x2 bias row + all y prep; own PSUM pool, closed before the
            # main pool claims all 8 banks. All y input DMAs issue first on
            # the SP ring so nothing queues behind output DMAs later.
            nchunks = m // prep_chunk
            with (
                tc.tile_pool(name="yraw_sb", bufs=nchunks) as yrp,
                tc.tile_pool(name="prep_sb", bufs=2) as psb,
                tc.tile_pool(name="prep_psum", bufs=2, space="PSUM") as pp,
            ):
                y_raws = []
                for jc in range(0, m, prep_chunk):
                    y_raw = yrp.tile([d, prep_chunk], F32, tag="y_raw")
                    nc.sync.dma_start(out=y_raw[:, :], in_=yT[:, jc : jc + prep_chunk])
                    y_raws.append(y_raw)

                for j0 in range(0, nsh, mm_n):
                    w = min(mm_n, nsh - j0)
                    ps = pp.tile([1, mm_n], F32, tag="x2ps")
                    nc.tensor.matmul(
                        ps[:, :w], mmi(neghalf[:, :]), mmi(xsq[:, j0 : j0 + w]),
                        start=True, stop=True,
                    )
                    nc.scalar.activation(
                        x2row[:, j0 : j0 + w], ps[:, :w], AF.Identity,
                        bias=los_sb[:, :],
                    )
                # transpose x2row chunks to per-partition bias cols [128, n_tiles]
                for i in range(n_tiles):
                    ps = pp.tile([128, 1], F32, tag="biasps")
                    nc.tensor.matmul(
                        ps[:, :], x2row[:, i * 128 : (i + 1) * 128], ones11[:, :],
                        start=True, stop=True,
                    )
                    nc.vector.tensor_copy(bias_sb[:, i : i + 1], ps[:, :])

                # ---- y prep, all upfront (no PE work inside the main sweep
                # -> no FIFO coupling, no mid-stream gap). Row ops split
                # across three engines: square on ACT, y2-hi copy on gpsimd,
                # y2-lo residual on DVE. DVE/gpsimd writes must start at
                # partition {0,32,64,96}: row d+1 (partition 65) goes via a
                # partition-0 tmp + DMA on the scalar ring. ----
                for jc in range(0, m, prep_chunk):
                    slc = slice(jc, jc + prep_chunk)
                    y_raw = y_raws[jc // prep_chunk]
                    nc.gpsimd.tensor_scalar_mul(
                        mmo(y_aug[0:d, slc]), y_raw[:, :], invl[:, :]
                    )
                    ysq = psb.tile([d, prep_chunk], F32, tag="ysq")
                    nc.scalar.activation(
                        mmo(ysq[:, :]), y_raw[:, :], AF.Square, scale=invl[:, :]
                    )
                    for j0 in range(0, prep_chunk, mm_n):
                        sl = slice(jc + j0, jc + j0 + mm_n)
                        ps = pp.tile([1, mm_n], F32, tag="y2ps")
                        nc.tensor.matmul(
                            ps[:, :], mmi(neghalf[:, :]), mmi(ysq[:, j0 : j0 + mm_n]),
                            start=True, stop=True,
                        )
                        nc.vector.tensor_copy(mmo(y_aug[d : d + 1, sl]), ps[:, :])
                        if use_f32r:
                            nc.vector.tensor_sub(
                                mmo(y2l_tmp[:, sl]), ps[:, :], y_aug[d : d + 1, sl],
                            )
                        else:
                            nc.vector.memset(y2l_tmp[:, sl], 0.0)
                    nc.scalar.dma_start(
                        out=mmo(y_aug[d + 1 : d + 2, slc]),
                        in_=mmo(y2l_tmp[:, slc]),
                    )

            # ---- main sweep: pure matmul -> exp -> 2 MiB SP-ring DMA ----
            with tc.tile_pool(name="main_psum", bufs=2, space="PSUM") as mp:
                for h in range(2):
                    # sweep x tiles over this half; 2 MiB out DMAs on SP ring
                    for i in range(n_tiles):
                        ot = opool.tile([128, half], F32, tag="ot")
                        for j2 in range(hc):
                            ps = mp.tile([128, n_chunk], F32, tag="mm")
                            for jj in range(n_sub):
                                col = h * half + j2 * n_chunk + jj * mm_n
                                nc.tensor.matmul(
                                    ps[:, jj * mm_n : (jj + 1) * mm_n],
                                    mmi(x_aug[:, i * 128 : (i + 1) * 128]),
                                    mmi(y_aug[:, col : col + mm_n]),
                                    start=True, stop=True,
                                )
                            nc.scalar.activation(
                                ot[:, j2 * n_chunk : (j2 + 1) * n_chunk],
                                ps[:, :], AF.Exp, bias=bias_sb[:, i : i + 1],
                            )
                        nc.sync.dma_start(
                            out=out[i * 128 : (i + 1) * 128, h * half : (h + 1) * half],
                            in_=ot[:, :],
                        )
    nc.finalize()
    return nc


_NC_CACHE = {}


def _get_nc():
    if "nc" not in _NC_CACHE:
        _NC_CACHE["nc"] = build_nc()
    return _NC_CACHE["nc"]


def stage_inputs(x, y, log_lengthscale, log_outputscale):
    x = np.ascontiguousarray(np.asarray(x, dtype=np.float32))
    y = np.ascontiguousarray(np.asarray(y, dtype=np.float32))
    lls = np.ascontiguousarray(np.asarray(log_lengthscale, dtype=np.float32))
    los = np.ascontiguousarray(np.asarray(log_outputscale, dtype=np.float32))

    yT = np.ascontiguousarray(y.T)  # [D, M]
    in_maps = []
    for c in range(N_CORES):
        xT_c = np.ascontiguousarray(x[c * NSH : (c + 1) * NSH].T)  # [D, NSH]
        in_maps.append(
            {"xT": xT_c, "yT": yT, "log_lengthscale": lls, "log_outputscale": los}
        )
    return in_maps


def kernel(x, y, log_lengthscale, log_outputscale):
    in_maps = stage_inputs(x, y, log_lengthscale, log_outputscale)
    res = run_bass_kernel_spmd(_get_nc(), in_maps, core_ids=list(range(N_CORES)))
    return np.concatenate([r["out"] for r in res.results], axis=0)
